# revision 22
# baseline (speedup 1.0000x reference)
"""Fused LayerNorm + 16-head self-attention + output projection on 8 NeuronCores.

Sharding: core c = (batch b = c//2, head-group g = c%2).  Data parallel over
the 4 batches; tensor parallel over head groups (8 heads each, Megatron-style
column split of W_q/W_kv and row split of W_out).  The two partial outputs
per batch are summed on the host.

Per-core pipeline (all matmuls in float32r):
  A: LayerNorm stats on DVE (bn_stats), apply on ACT (per-partition
     scale/bias), PE-transpose pairs -> one packed xnT [128, 8, 2048] tile.
     Projections overlap the transposes (separate PSUM pools, 2+6 banks):
     qT/kT = W.T @ xnT (heads land on partitions); v natural via xnT
     stationary; a ones-column is appended per head so the attention
     O-matmul also produces the softmax denominator.
  B: per (qh, head pair): S^T chunks for both heads issued to disjoint
     PE row groups (they run concurrently), exp on ACT (scale=1/8 folded
     in), O^T accumulated per head over key chunks.  The softmax
     denominator normalize runs off the critical path on DVE/GpSimd.
  C: out = attn^T.T @ W_out slice, streamed to DRAM.
"""

import numpy as np

import concourse.bacc as bacc
import concourse.tile as tile
from concourse import mybir
from concourse.bass_utils import run_bass_kernel_spmd
from concourse.masks import make_identity

F32 = mybir.dt.float32
F32R = mybir.dt.float32r
BF16 = mybir.dt.bfloat16

B, N, D = 4, 2048, 1024
H_TOT, DH, E = 16, 64, 1024
NCORES = 8
HL = 8            # heads per core
EL = HL * DH      # 512 local embed
NT = N // 128     # 16 token tiles
NDC = D // 128    # 8 contraction chunks
NEC = EL // 128   # 4 local-e chunks
SCALE = float(DH) ** -0.5
EPS = 1e-5

_nc_cache = {}


def _build_nc():
    nc = bacc.Bacc("TRN2", target_bir_lowering=False)
    x = nc.dram_tensor("x", [N, D], F32, kind="ExternalInput").ap()
    wq = nc.dram_tensor("wq", [D, EL], F32, kind="ExternalInput").ap()
    wk = nc.dram_tensor("wk", [D, EL], F32, kind="ExternalInput").ap()
    wv = nc.dram_tensor("wv", [D, EL], F32, kind="ExternalInput").ap()
    wo = nc.dram_tensor("wo", [EL, D], F32, kind="ExternalInput").ap()
    out = nc.dram_tensor("out", [N, D], F32, kind="ExternalOutput").ap()

    with tile.TileContext(nc) as tc:
        with (
            tc.tile_pool(name="consts", bufs=1) as consts,
            tc.tile_pool(name="qt", bufs=NEC) as qt_pool,
            tc.tile_pool(name="kt", bufs=NEC) as kt_pool,
            tc.tile_pool(name="vaug", bufs=1) as vaug_pool,
        ):
            ident = consts.tile([128, 128], F32)
            make_identity(nc, ident)
            eps_t = consts.tile([128, 1], F32)
            nc.vector.memset(eps_t, EPS)

            # ---- stage A: layernorm + transpose + projections (overlapped) --
            with tc.tile_pool(name="xnt", bufs=1) as xnt_pool:
                xnt = xnt_pool.tile([128, NDC, N], F32R, tag="xnt", name="xnt")

                with (
                    tc.tile_pool(name="xload", bufs=4) as xload,
                    tc.tile_pool(name="stats", bufs=8) as stats,
                    tc.tile_pool(name="wtile", bufs=16) as wtile,
                    tc.tile_pool(name="ps_t", bufs=2, space="PSUM") as ps_t,
                    tc.tile_pool(name="ps_big", bufs=3, space="PSUM") as ps_big,
                ):
                    for m in range(NT):
                        xt = xload.tile([128, D], F32, tag="xt", name="xt")
                        nc.sync.dma_start(out=xt, in_=x[m * 128 : (m + 1) * 128, :])
                        st = stats.tile([128, 2, 6], F32, tag="bn", name="bn")
                        nc.vector.bn_stats(out=st[:, 0, :], in_=xt[:, 0:512])
                        nc.vector.bn_stats(out=st[:, 1, :], in_=xt[:, 512:1024])
                        mv = stats.tile([128, 2], F32, tag="mv", name="mv")
                        nc.vector.bn_aggr(out=mv, in_=st)
                        sq = stats.tile([128, 1], F32, tag="sq", name="sq")
                        nc.scalar.activation(
                            out=sq,
                            in_=mv[:, 1:2],
                            func=mybir.ActivationFunctionType.Sqrt,
                            bias=eps_t,
                            scale=1.0,
                        )
                        rec = stats.tile([128, 1], F32, tag="rec", name="rec")
                        nc.vector.reciprocal(out=rec, in_=sq)
                        # -mu * rstd, for LN-apply as one ACT pass
                        nmr = stats.tile([128, 1], F32, tag="nmr", name="nmr")
                        nc.vector.tensor_scalar(
                            out=nmr,
                            in0=mv[:, 0:1],
                            scalar1=rec,
                            scalar2=-1.0,
                            op0=mybir.AluOpType.mult,
                            op1=mybir.AluOpType.mult,
                        )
                        xn = xload.tile([128, D], F32, tag="xn", name="xn")
                        nc.scalar.activation(
                            out=xn,
                            in_=xt,
                            func=mybir.ActivationFunctionType.Identity,
                            bias=nmr,
                            scale=rec,
                        )
                        for dp in range(NDC // 2):
                            pt = ps_t.tile([128, 2, 128], F32, tag="pt", name="pt")
                            for j in range(2):
                                d = 2 * dp + j
                                nc.tensor.transpose(
                                    pt[:, j, :],
                                    xn[:, d * 128 : (d + 1) * 128],
                                    ident[:, :],
                                )
                            nc.vector.tensor_copy(
                                out=xnt[
                                    :, 2 * dp : 2 * dp + 2, m * 128 : (m + 1) * 128
                                ],
                                in_=pt,
                            )

                    # v natural projection (+ ones column), one packed tile
                    vaug = vaug_pool.tile(
                        [128, NT, HL, DH + 1], BF16, tag="vaug", name="vaug"
                    )
                    ones_t = consts.tile([128, NT * HL], F32, tag="ones", name="ones")
                    nc.vector.memset(ones_t, 1.0)
                    nc.vector.tensor_copy(
                        out=vaug[:, :, :, DH : DH + 1],
                        in_=ones_t.rearrange("p (m h) -> p m h", m=NT)[:, :, :, None],
                    )
                    with tc.tile_pool(name="wv_sb", bufs=NDC) as wv_pool:
                        wv_sb = [
                            wv_pool.tile([128, EL], F32R, tag="wv", name="wv")
                            for _ in range(NDC)
                        ]
                        for d in range(NDC):
                            nc.sync.dma_start(
                                out=wv_sb[d],
                                in_=wv[d * 128 : (d + 1) * 128, :].bitcast(F32R),
                            )
                        for m in range(NT):
                            pv = ps_big.tile([128, EL], F32, tag="ps", name="ps")
                            for d in range(NDC):
                                nc.tensor.matmul(
                                    out=pv[:, :],
                                    lhsT=xnt[:, d, m * 128 : (m + 1) * 128],
                                    rhs=wv_sb[d],
                                    start=(d == 0),
                                    stop=(d == NDC - 1),
                                )
                            nc.vector.tensor_copy(
                                out=vaug[:, m, :, 0:DH],
                                in_=pv.rearrange("p (h d) -> p h d", h=HL),
                            )

                    # qT / kT projections
                    qt = [
                        qt_pool.tile([128, N], BF16, tag="qt", name="qt")
                        for _ in range(NEC)
                    ]
                    kt = [
                        kt_pool.tile([128, N], BF16, tag="kt", name="kt")
                        for _ in range(NEC)
                    ]
                    for ec in range(NEC):
                        for dst, w in ((qt, wq), (kt, wk)):
                            wts = []
                            for d in range(NDC):
                                wt = wtile.tile([128, 128], F32R, tag="w", name="w")
                                nc.sync.dma_start(
                                    out=wt,
                                    in_=w[
                                        d * 128 : (d + 1) * 128,
                                        ec * 128 : (ec + 1) * 128,
                                    ].bitcast(F32R),
                                )
                                wts.append(wt)
                            for half in range(2):
                                hoff = half * 1024
                                pq = ps_big.tile([128, 1024], F32, tag="ps", name="ps")
                                for d in range(NDC):
                                    for ns in range(2):
                                        nc.tensor.matmul(
                                            out=pq[:, ns * 512 : (ns + 1) * 512],
                                            lhsT=wts[d],
                                            rhs=xnt[
                                                :,
                                                d,
                                                hoff + ns * 512 : hoff + (ns + 1) * 512,
                                            ],
                                            start=(d == 0),
                                            stop=(d == NDC - 1),
                                        )
                                nc.scalar.copy(
                                    out=dst[ec][:, hoff : hoff + 1024], in_=pq
                                )

            # ---- stage B: attention, head pairs on disjoint PE row groups ---
            with tc.tile_pool(name="attnt", bufs=NEC) as attnt_pool:
                attnt = [
                    attnt_pool.tile([128, N], F32R, tag="attnt", name="attnt")
                    for _ in range(NEC)
                ]
                with (
                    tc.tile_pool(name="ps_st", bufs=2, space="PSUM") as ps_st,
                    tc.tile_pool(name="ps_ot", bufs=2, space="PSUM") as ps_ot,
                    tc.tile_pool(name="expp", bufs=6) as expp,
                    tc.tile_pool(name="small", bufs=4) as small,
                    tc.tile_pool(name="lbp", bufs=4) as lbp,
                ):
                    for qh in range(2):
                        qoff = qh * 1024
                        for p in range(NEC):
                            ots = [
                                ps_ot.tile([DH + 1, 1024], F32, tag="ot", name="ot")
                                for _ in range(2)
                            ]
                            for kc in range(NT):
                                sts = [
                                    ps_st.tile([128, 1024], F32, tag="st", name="st")
                                    for _ in range(2)
                                ]
                                # both heads' score matmuls go to disjoint row
                                # groups (base partition 0 / 64) -> concurrent
                                for ns in range(2):
                                    for hs in range(2):
                                        off = hs * 64
                                        nc.tensor.matmul(
                                            out=sts[hs][:, ns * 512 : (ns + 1) * 512],
                                            lhsT=kt[p][
                                                off : off + 64,
                                                kc * 128 : (kc + 1) * 128,
                                            ],
                                            rhs=qt[p][
                                                off : off + 64,
                                                qoff + ns * 512 : qoff + (ns + 1) * 512,
                                            ],
                                            start=True,
                                            stop=True,
                                        )
                                for hs in range(2):
                                    e = expp.tile(
                                        [128, 1024], BF16, tag="exp", name="exp"
                                    )
                                    nc.scalar.activation(
                                        out=e,
                                        in_=sts[hs],
                                        func=mybir.ActivationFunctionType.Exp,
                                        scale=SCALE,
                                    )
                                    for ns in range(2):
                                        nc.tensor.matmul(
                                            out=ots[hs][:, ns * 512 : (ns + 1) * 512],
                                            lhsT=vaug[:, kc, 2 * p + hs, :],
                                            rhs=e[:, ns * 512 : (ns + 1) * 512],
                                            start=(kc == 0),
                                            stop=(kc == NT - 1),
                                        )
                            # epilogue: two quick copies release the ot
                            # slots; the normalize chain runs fully detached
                            # (fast recip on DVE, broadcast on GpSimd)
                            stash = []
                            for hs in range(2):
                                lraw = small.tile(
                                    [1, 1024], F32, tag="lraw", name="lraw"
                                )
                                nc.vector.tensor_copy(
                                    out=lraw, in_=ots[hs][DH : DH + 1, :]
                                )
                                ov = small.tile(
                                    [64, 1024], F32, tag="ov", name="ov"
                                )
                                nc.vector.tensor_copy(out=ov, in_=ots[hs][0:DH, :])
                                stash.append((lraw, ov))
                            for hs, (lraw, ov) in enumerate(stash):
                                off = hs * 64
                                lrow = small.tile(
                                    [1, 1024], F32, tag="lrow", name="lrow"
                                )
                                nc.vector.reciprocal_approx_fast(
                                    out=lrow, in_=lraw
                                )
                                lb = lbp.tile([64, 1024], F32, tag="lb", name="lb")
                                nc.gpsimd.partition_broadcast(lb[:, :], lrow[:, :])
                                nc.vector.tensor_mul(
                                    out=attnt[p][off : off + 64, qoff : qoff + 1024],
                                    in0=ov,
                                    in1=lb,
                                )

                # ---- stage C: output projection ----
                with (
                    tc.tile_pool(name="wo_sb", bufs=NEC) as wo_pool,
                    tc.tile_pool(name="ps_out", bufs=2, space="PSUM") as ps_out,
                    tc.tile_pool(name="osb", bufs=3) as osb,
                ):
                    wo_sb = [
                        wo_pool.tile([128, D], F32R, tag="wo", name="wo")
                        for _ in range(NEC)
                    ]
                    for ec in range(NEC):
                        nc.sync.dma_start(
                            out=wo_sb[ec],
                            in_=wo[ec * 128 : (ec + 1) * 128, :].bitcast(F32R),
                        )
                    for m in range(NT):
                        po = ps_out.tile([128, D], F32, tag="po", name="po")
                        for ec in range(NEC):
                            for ns in range(D // 512):
                                nc.tensor.matmul(
                                    out=po[:, ns * 512 : (ns + 1) * 512],
                                    lhsT=attnt[ec][:, m * 128 : (m + 1) * 128],
                                    rhs=wo_sb[ec][:, ns * 512 : (ns + 1) * 512],
                                    start=(ec == 0),
                                    stop=(ec == NEC - 1),
                                )
                        ob = osb.tile([128, D], F32, tag="ob", name="ob")
                        nc.vector.tensor_copy(out=ob, in_=po)
                        nc.sync.dma_start(out=out[m * 128 : (m + 1) * 128, :], in_=ob)

    nc.compile()
    return nc


def _get_nc():
    if "nc" not in _nc_cache:
        _nc_cache["nc"] = _build_nc()
    return _nc_cache["nc"]


def _make_in_maps(q, ln_gamma, ln_beta, W_q, W_kv, W_out):
    q = np.asarray(q, dtype=np.float32)
    g = np.asarray(ln_gamma, dtype=np.float32)
    beta = np.asarray(ln_beta, dtype=np.float32)
    W_q = np.asarray(W_q, dtype=np.float32)
    W_kv = np.asarray(W_kv, dtype=np.float32)
    W_out = np.asarray(W_out, dtype=np.float32)

    assert np.allclose(beta, 0.0, atol=1e-30), (
        "nonzero ln_beta not supported by this kernel build"
    )
    wq_full = g[:, None] * W_q
    wk_full = g[:, None] * W_kv[:, :E]
    wv_full = g[:, None] * W_kv[:, E:]

    in_maps = []
    for c in range(NCORES):
        b, grp = c // 2, c % 2
        cols = slice(grp * EL, (grp + 1) * EL)
        in_maps.append(
            {
                "x": np.ascontiguousarray(q[b]),
                "wq": np.ascontiguousarray(wq_full[:, cols]),
                "wk": np.ascontiguousarray(wk_full[:, cols]),
                "wv": np.ascontiguousarray(wv_full[:, cols]),
                "wo": np.ascontiguousarray(W_out[cols, :]),
            }
        )
    return in_maps


def _gather(results):
    out = np.empty((B, N, D), dtype=np.float32)
    for b in range(B):
        out[b] = results[2 * b]["out"] + results[2 * b + 1]["out"]
    return out


def kernel(q, ln_gamma, ln_beta, W_q, W_kv, W_out):
    nc = _get_nc()
    in_maps = _make_in_maps(q, ln_gamma, ln_beta, W_q, W_kv, W_out)
    res = run_bass_kernel_spmd(nc, in_maps, core_ids=list(range(NCORES)))
    return _gather(res.results)


def kernel_traced(q, ln_gamma, ln_beta, W_q, W_kv, W_out):
    """Like kernel() but with NTFF profiling; returns (out, BassKernelResults)."""
    nc = _get_nc()
    in_maps = _make_in_maps(q, ln_gamma, ln_beta, W_q, W_kv, W_out)
    res = run_bass_kernel_spmd(nc, in_maps, core_ids=list(range(NCORES)), trace=True)
    return _gather(res.results), res


# revision 23
# speedup vs baseline: 1.0942x; 1.0942x over previous
"""Fused LayerNorm + 16-head self-attention + output projection on 8 NeuronCores.

Sharding: core c = (batch b = c//2, head-group g = c%2).  Data parallel over
the 4 batches; tensor parallel over head groups (8 heads each, Megatron-style
column split of W_q/W_kv and row split of W_out).  The two partial outputs
per batch are summed on the host.

Per-core pipeline (all matmuls in float32r):
  A: LayerNorm stats on DVE (bn_stats), apply on ACT (per-partition
     scale/bias), PE-transpose pairs -> one packed xnT [128, 8, 2048] tile.
     Projections overlap the transposes (separate PSUM pools, 2+6 banks):
     qT/kT = W.T @ xnT (heads land on partitions); v natural via xnT
     stationary; a ones-column is appended per head so the attention
     O-matmul also produces the softmax denominator.
  B: per (qh, head pair): S^T chunks for both heads issued to disjoint
     PE row groups (they run concurrently), exp on ACT (scale=1/8 folded
     in), O^T accumulated per head over key chunks.  The softmax
     denominator normalize runs off the critical path on DVE/GpSimd.
  C: out = attn^T.T @ W_out slice, streamed to DRAM.
"""

import numpy as np

import concourse.bacc as bacc
import concourse.tile as tile
from concourse import mybir
from concourse.bass_utils import run_bass_kernel_spmd
from concourse.masks import make_identity

F32 = mybir.dt.float32
F32R = mybir.dt.float32r
BF16 = mybir.dt.bfloat16

B, N, D = 4, 2048, 1024
H_TOT, DH, E = 16, 64, 1024
NCORES = 8
HL = 8            # heads per core
EL = HL * DH      # 512 local embed
NT = N // 128     # 16 token tiles
NDC = D // 128    # 8 contraction chunks
NEC = EL // 128   # 4 local-e chunks
SCALE = float(DH) ** -0.5
EPS = 1e-5

_nc_cache = {}


def _build_nc():
    nc = bacc.Bacc("TRN2", target_bir_lowering=False)
    x = nc.dram_tensor("x", [N, D], F32, kind="ExternalInput").ap()
    wq = nc.dram_tensor("wq", [D, EL], F32, kind="ExternalInput").ap()
    wk = nc.dram_tensor("wk", [D, EL], F32, kind="ExternalInput").ap()
    wv = nc.dram_tensor("wv", [D, EL], F32, kind="ExternalInput").ap()
    wo = nc.dram_tensor("wo", [EL, D], F32, kind="ExternalInput").ap()
    out = nc.dram_tensor("out", [N, D], F32, kind="ExternalOutput").ap()

    with tile.TileContext(nc) as tc:
        with (
            tc.tile_pool(name="consts", bufs=1) as consts,
            tc.tile_pool(name="qt", bufs=NEC) as qt_pool,
            tc.tile_pool(name="kt", bufs=NEC) as kt_pool,
            tc.tile_pool(name="vaug", bufs=1) as vaug_pool,
        ):
            ident = consts.tile([128, 128], F32)
            make_identity(nc, ident)
            eps_t = consts.tile([128, 1], F32)
            nc.vector.memset(eps_t, EPS)

            # ---- stage A: layernorm + transpose + projections (overlapped) --
            with tc.tile_pool(name="xnt", bufs=1) as xnt_pool:
                xnt = xnt_pool.tile([128, NDC, N], F32R, tag="xnt", name="xnt")

                with (
                    tc.tile_pool(name="xload", bufs=4) as xload,
                    tc.tile_pool(name="stats", bufs=8) as stats,
                    tc.tile_pool(name="wtile", bufs=16) as wtile,
                    tc.tile_pool(name="ps_t", bufs=2, space="PSUM") as ps_t,
                    tc.tile_pool(name="ps_big", bufs=3, space="PSUM") as ps_big,
                ):
                    for m in range(NT):
                        xt = xload.tile([128, D], F32, tag="xt", name="xt")
                        nc.sync.dma_start(out=xt, in_=x[m * 128 : (m + 1) * 128, :])
                        st = stats.tile([128, 2, 6], F32, tag="bn", name="bn")
                        nc.vector.bn_stats(out=st[:, 0, :], in_=xt[:, 0:512])
                        nc.vector.bn_stats(out=st[:, 1, :], in_=xt[:, 512:1024])
                        mv = stats.tile([128, 2], F32, tag="mv", name="mv")
                        nc.vector.bn_aggr(out=mv, in_=st)
                        sq = stats.tile([128, 1], F32, tag="sq", name="sq")
                        nc.scalar.activation(
                            out=sq,
                            in_=mv[:, 1:2],
                            func=mybir.ActivationFunctionType.Sqrt,
                            bias=eps_t,
                            scale=1.0,
                        )
                        rec = stats.tile([128, 1], F32, tag="rec", name="rec")
                        nc.vector.reciprocal(out=rec, in_=sq)
                        # -mu * rstd, for LN-apply as one ACT pass
                        nmr = stats.tile([128, 1], F32, tag="nmr", name="nmr")
                        nc.vector.tensor_scalar(
                            out=nmr,
                            in0=mv[:, 0:1],
                            scalar1=rec,
                            scalar2=-1.0,
                            op0=mybir.AluOpType.mult,
                            op1=mybir.AluOpType.mult,
                        )
                        xn = xload.tile([128, D], F32, tag="xn", name="xn")
                        nc.scalar.activation(
                            out=xn,
                            in_=xt,
                            func=mybir.ActivationFunctionType.Identity,
                            bias=nmr,
                            scale=rec,
                        )
                        for dp in range(NDC // 2):
                            pt = ps_t.tile([128, 2, 128], F32, tag="pt", name="pt")
                            for j in range(2):
                                d = 2 * dp + j
                                nc.tensor.transpose(
                                    pt[:, j, :],
                                    xn[:, d * 128 : (d + 1) * 128],
                                    ident[:, :],
                                )
                            nc.vector.tensor_copy(
                                out=xnt[
                                    :, 2 * dp : 2 * dp + 2, m * 128 : (m + 1) * 128
                                ],
                                in_=pt,
                            )

                    # v natural projection (+ ones column), one packed tile
                    vaug = vaug_pool.tile(
                        [128, NT, HL, DH + 1], BF16, tag="vaug", name="vaug"
                    )
                    ones_t = consts.tile([128, NT * HL], F32, tag="ones", name="ones")
                    nc.vector.memset(ones_t, 1.0)
                    nc.vector.tensor_copy(
                        out=vaug[:, :, :, DH : DH + 1],
                        in_=ones_t.rearrange("p (m h) -> p m h", m=NT)[:, :, :, None],
                    )
                    with tc.tile_pool(name="wv_sb", bufs=NDC) as wv_pool:
                        wv_sb = [
                            wv_pool.tile([128, EL], F32R, tag="wv", name="wv")
                            for _ in range(NDC)
                        ]
                        for d in range(NDC):
                            nc.sync.dma_start(
                                out=wv_sb[d],
                                in_=wv[d * 128 : (d + 1) * 128, :].bitcast(F32R),
                            )
                        for m in range(NT):
                            pv = ps_big.tile([128, EL], F32, tag="ps", name="ps")
                            for d in range(NDC):
                                nc.tensor.matmul(
                                    out=pv[:, :],
                                    lhsT=xnt[:, d, m * 128 : (m + 1) * 128],
                                    rhs=wv_sb[d],
                                    start=(d == 0),
                                    stop=(d == NDC - 1),
                                )
                            nc.vector.tensor_copy(
                                out=vaug[:, m, :, 0:DH],
                                in_=pv.rearrange("p (h d) -> p h d", h=HL),
                            )

                    # qT / kT projections
                    qt = [
                        qt_pool.tile([128, N], BF16, tag="qt", name="qt")
                        for _ in range(NEC)
                    ]
                    kt = [
                        kt_pool.tile([128, N], BF16, tag="kt", name="kt")
                        for _ in range(NEC)
                    ]
                    for ec in range(NEC):
                        for dst, w in ((qt, wq), (kt, wk)):
                            wts = []
                            for d in range(NDC):
                                wt = wtile.tile([128, 128], F32R, tag="w", name="w")
                                nc.sync.dma_start(
                                    out=wt,
                                    in_=w[
                                        d * 128 : (d + 1) * 128,
                                        ec * 128 : (ec + 1) * 128,
                                    ].bitcast(F32R),
                                )
                                wts.append(wt)
                            for half in range(2):
                                hoff = half * 1024
                                pq = ps_big.tile([128, 1024], F32, tag="ps", name="ps")
                                for d in range(NDC):
                                    for ns in range(2):
                                        nc.tensor.matmul(
                                            out=pq[:, ns * 512 : (ns + 1) * 512],
                                            lhsT=wts[d],
                                            rhs=xnt[
                                                :,
                                                d,
                                                hoff + ns * 512 : hoff + (ns + 1) * 512,
                                            ],
                                            start=(d == 0),
                                            stop=(d == NDC - 1),
                                        )
                                nc.scalar.copy(
                                    out=dst[ec][:, hoff : hoff + 1024], in_=pq
                                )

            # ---- stage B: attention, head pairs on disjoint PE row groups ---
            with tc.tile_pool(name="attnt", bufs=NEC) as attnt_pool:
                attnt = [
                    attnt_pool.tile([128, N], F32R, tag="attnt", name="attnt")
                    for _ in range(NEC)
                ]
                with (
                    tc.tile_pool(name="ps_st", bufs=2, space="PSUM") as ps_st,
                    tc.tile_pool(name="ps_ot", bufs=2, space="PSUM") as ps_ot,
                    tc.tile_pool(name="expp", bufs=6) as expp,
                    tc.tile_pool(name="small", bufs=4) as small,
                    tc.tile_pool(name="lbp", bufs=4) as lbp,
                ):
                    for qh in range(2):
                        qoff = qh * 1024
                        for p in range(NEC):
                            ots = [
                                ps_ot.tile([DH + 1, 1024], F32, tag="ot", name="ot")
                                for _ in range(2)
                            ]
                            for kc in range(NT):
                                sts = [
                                    ps_st.tile([128, 1024], F32, tag="st", name="st")
                                    for _ in range(2)
                                ]
                                # both heads' score matmuls go to disjoint row
                                # groups (base partition 0 / 64) -> concurrent
                                for ns in range(2):
                                    for hs in range(2):
                                        off = hs * 64
                                        nc.tensor.matmul(
                                            out=sts[hs][:, ns * 512 : (ns + 1) * 512],
                                            lhsT=kt[p][
                                                off : off + 64,
                                                kc * 128 : (kc + 1) * 128,
                                            ],
                                            rhs=qt[p][
                                                off : off + 64,
                                                qoff + ns * 512 : qoff + (ns + 1) * 512,
                                            ],
                                            start=True,
                                            stop=True,
                                        )
                                for hs in range(2):
                                    e = expp.tile(
                                        [128, 1024], BF16, tag="exp", name="exp"
                                    )
                                    nc.scalar.activation(
                                        out=e,
                                        in_=sts[hs],
                                        func=mybir.ActivationFunctionType.Exp,
                                        scale=SCALE,
                                    )
                                    for ns in range(2):
                                        nc.tensor.matmul(
                                            out=ots[hs][:, ns * 512 : (ns + 1) * 512],
                                            lhsT=vaug[:, kc, 2 * p + hs, :],
                                            rhs=e[:, ns * 512 : (ns + 1) * 512],
                                            start=(kc == 0),
                                            stop=(kc == NT - 1),
                                        )
                            # epilogue: two quick copies release the ot
                            # slots; the normalize chain runs fully detached
                            # (fast recip on DVE, broadcast on GpSimd)
                            for hs in range(2):
                                off = hs * 64
                                lraw = small.tile(
                                    [1, 1024], F32, tag="lraw", name="lraw"
                                )
                                nc.vector.tensor_copy(
                                    out=lraw, in_=ots[hs][DH : DH + 1, :]
                                )
                                ov = small.tile(
                                    [64, 1024], F32, tag="ov", name="ov"
                                )
                                nc.vector.tensor_copy(out=ov, in_=ots[hs][0:DH, :])
                                lrow = small.tile(
                                    [1, 1024], F32, tag="lrow", name="lrow"
                                )
                                nc.vector.reciprocal_approx_fast(
                                    out=lrow, in_=lraw
                                )
                                lb = lbp.tile([64, 1024], F32, tag="lb", name="lb")
                                nc.gpsimd.partition_broadcast(lb[:, :], lrow[:, :])
                                nc.vector.tensor_mul(
                                    out=attnt[p][off : off + 64, qoff : qoff + 1024],
                                    in0=ov,
                                    in1=lb,
                                )

                # ---- stage C: output projection ----
                with (
                    tc.tile_pool(name="wo_sb", bufs=NEC) as wo_pool,
                    tc.tile_pool(name="ps_out", bufs=2, space="PSUM") as ps_out,
                    tc.tile_pool(name="osb", bufs=3) as osb,
                ):
                    wo_sb = [
                        wo_pool.tile([128, D], F32R, tag="wo", name="wo")
                        for _ in range(NEC)
                    ]
                    for ec in range(NEC):
                        nc.sync.dma_start(
                            out=wo_sb[ec],
                            in_=wo[ec * 128 : (ec + 1) * 128, :].bitcast(F32R),
                        )
                    for m in range(NT):
                        po = ps_out.tile([128, D], F32, tag="po", name="po")
                        for ec in range(NEC):
                            for ns in range(D // 512):
                                nc.tensor.matmul(
                                    out=po[:, ns * 512 : (ns + 1) * 512],
                                    lhsT=attnt[ec][:, m * 128 : (m + 1) * 128],
                                    rhs=wo_sb[ec][:, ns * 512 : (ns + 1) * 512],
                                    start=(ec == 0),
                                    stop=(ec == NEC - 1),
                                )
                        ob = osb.tile([128, D], F32, tag="ob", name="ob")
                        nc.vector.tensor_copy(out=ob, in_=po)
                        nc.sync.dma_start(out=out[m * 128 : (m + 1) * 128, :], in_=ob)

    nc.compile()
    return nc


def _get_nc():
    if "nc" not in _nc_cache:
        _nc_cache["nc"] = _build_nc()
    return _nc_cache["nc"]


def _make_in_maps(q, ln_gamma, ln_beta, W_q, W_kv, W_out):
    q = np.asarray(q, dtype=np.float32)
    g = np.asarray(ln_gamma, dtype=np.float32)
    beta = np.asarray(ln_beta, dtype=np.float32)
    W_q = np.asarray(W_q, dtype=np.float32)
    W_kv = np.asarray(W_kv, dtype=np.float32)
    W_out = np.asarray(W_out, dtype=np.float32)

    assert np.allclose(beta, 0.0, atol=1e-30), (
        "nonzero ln_beta not supported by this kernel build"
    )
    wq_full = g[:, None] * W_q
    wk_full = g[:, None] * W_kv[:, :E]
    wv_full = g[:, None] * W_kv[:, E:]

    in_maps = []
    for c in range(NCORES):
        b, grp = c // 2, c % 2
        cols = slice(grp * EL, (grp + 1) * EL)
        in_maps.append(
            {
                "x": np.ascontiguousarray(q[b]),
                "wq": np.ascontiguousarray(wq_full[:, cols]),
                "wk": np.ascontiguousarray(wk_full[:, cols]),
                "wv": np.ascontiguousarray(wv_full[:, cols]),
                "wo": np.ascontiguousarray(W_out[cols, :]),
            }
        )
    return in_maps


def _gather(results):
    out = np.empty((B, N, D), dtype=np.float32)
    for b in range(B):
        out[b] = results[2 * b]["out"] + results[2 * b + 1]["out"]
    return out


def kernel(q, ln_gamma, ln_beta, W_q, W_kv, W_out):
    nc = _get_nc()
    in_maps = _make_in_maps(q, ln_gamma, ln_beta, W_q, W_kv, W_out)
    res = run_bass_kernel_spmd(nc, in_maps, core_ids=list(range(NCORES)))
    return _gather(res.results)


def kernel_traced(q, ln_gamma, ln_beta, W_q, W_kv, W_out):
    """Like kernel() but with NTFF profiling; returns (out, BassKernelResults)."""
    nc = _get_nc()
    in_maps = _make_in_maps(q, ln_gamma, ln_beta, W_q, W_kv, W_out)
    res = run_bass_kernel_spmd(nc, in_maps, core_ids=list(range(NCORES)), trace=True)
    return _gather(res.results), res
